# revision 15
# baseline (speedup 1.0000x reference)
"""Allegro-style GNN message passing on 8 TRN2 NeuronCores.

Strategy (edges sharded contiguously across 8 cores):
  - Per core 20000 edges, sorted by edge_center, bucketed into 40 blocks of
    128 atoms, each block padded to T_B=5 tiles of 128 edges (E_pad=25600).
  - Feature-major (transposed) activation layout [feat<=128, edges].
  - segment_sum  = per-tile one-hot matmul (PE) accumulating per-block PSUM
    slabs -> bf16 local table [5120, 256] -> chunked AllReduce (bf16).
  - gather       = per-tile one-hot matmul against the AllReduced table slab.
  - One-hot matrices built on-chip from a tiny index table (no index DMA).
  - W_proj folded into the latent-MLP first layers on host (acc0 never
    re-read on device); tensor features loaded raw once, TP weights applied
    on-chip via scalar_tensor_tensor / weighted reduction matrices.
  - MLPs in bf16 on the TensorEngine with fp32 PSUM accumulation.
Host does layout prep (sort/pad/transpose/cast) + unshard only.
"""

import sys
import math

sys.path.insert(0, "/opt/trn_rl_repo")

import numpy as np
import ml_dtypes

import concourse.bass as bass
import concourse.bacc as bacc
from concourse import tile
import concourse.mybir as mybir
from concourse.bass_utils import run_bass_kernel_spmd

BF = mybir.dt.bfloat16
F32 = mybir.dt.float32
BF_NP = ml_dtypes.bfloat16

# problem constants
E = 160000
N_ATOMS = 5000
C = 64
NS = 128
SCAL_IN = 64
HID = 256
NORM = 1.0 / math.sqrt(32.0)
INV_SQRT3 = 1.0 / math.sqrt(3.0)

N_CORES = 8
E_LOC = E // N_CORES          # 20000
ABLK = 128                    # atoms per block
T_B = 5                       # tiles (of 128 edges) per block
GRP = 4                       # tiles per matmul group (N=512)
N_CHUNK = 4                   # AllReduce chunks

# route the off-critical ov-chain elementwise ops to GPSIMD (Pool engine)
GPS_ELEMWISE = True


def build_graph(n_cores=N_CORES, n_blk=40, t_b=T_B):
    """Build the SPMD Bass graph (same graph runs on every core)."""
    NT = n_blk * t_b              # edge tiles
    E_PAD = NT * 128
    A = n_blk * ABLK              # padded atom count
    assert NT % GRP == 0
    NG = NT // GRP
    GW = GRP * 128
    assert n_blk % N_CHUNK == 0
    CB = n_blk // N_CHUNK         # blocks per AR chunk

    nc = bacc.Bacc("TRN2", target_bir_lowering=False, debug=False,
                   num_devices=n_cores)

    # ---- I/O ----
    seT = nc.dram_tensor("seT", [SCAL_IN, E_PAD], BF, kind="ExternalInput")
    tfT = nc.dram_tensor("tfT", [2, 128, E_PAD], BF, kind="ExternalInput")
    basis_pm = nc.dram_tensor("basis_pm", [128, NT, 4], BF, kind="ExternalInput")
    idxcol_h = nc.dram_tensor("idxcol", [128, NT], BF, kind="ExternalInput")
    iota_tile_h = nc.dram_tensor("iota_tile", [128, 512], BF, kind="ExternalInput")
    eye_h = nc.dram_tensor("eye", [128, 128], BF, kind="ExternalInput")
    redmat_h = nc.dram_tensor("redmat", [128, 64], BF, kind="ExternalInput")
    redmat_wa_h = nc.dram_tensor("redmat_wa", [128, 64], BF, kind="ExternalInput")
    redmat_wb_h = nc.dram_tensor("redmat_wb", [128, 64], BF, kind="ExternalInput")
    w2col_h = nc.dram_tensor("w2col", [128, 1], F32, kind="ExternalInput")
    w3col_h = nc.dram_tensor("w3col", [128, 1], F32, kind="ExternalInput")
    wproj_h = nc.dram_tensor("wproj", [64, 256], BF, kind="ExternalInput")
    # lat0: W1se [64,256] (W_proj folded), W1b [64,256]; W2/W3 [256,256]
    l0w1se_h = nc.dram_tensor("l0w1se", [64, 256], BF, kind="ExternalInput")
    l0w1b_h = nc.dram_tensor("l0w1b", [64, 256], BF, kind="ExternalInput")
    l0w2a_h = nc.dram_tensor("l0w2a", [128, 256], BF, kind="ExternalInput")
    l0w2b_h = nc.dram_tensor("l0w2b", [128, 256], BF, kind="ExternalInput")
    l0w3a_h = nc.dram_tensor("l0w3a", [128, 256], BF, kind="ExternalInput")
    l0w3b_h = nc.dram_tensor("l0w3b", [128, 256], BF, kind="ExternalInput")
    # lat1: W1se [64,256], W1b [128,256] (acc1), W1c [64,256] (ol)
    l1w1se_h = nc.dram_tensor("l1w1se", [64, 256], BF, kind="ExternalInput")
    l1w1b_h = nc.dram_tensor("l1w1b", [128, 256], BF, kind="ExternalInput")
    l1w1c_h = nc.dram_tensor("l1w1c", [64, 256], BF, kind="ExternalInput")
    l1w2a_h = nc.dram_tensor("l1w2a", [128, 256], BF, kind="ExternalInput")
    l1w2b_h = nc.dram_tensor("l1w2b", [128, 256], BF, kind="ExternalInput")
    l1w3a_h = nc.dram_tensor("l1w3a", [128, 128], BF, kind="ExternalInput")
    l1w3b_h = nc.dram_tensor("l1w3b", [128, 128], BF, kind="ExternalInput")

    acc0T = nc.dram_tensor("acc0T", [128, E_PAD], BF, kind="ExternalOutput")
    acc1T = nc.dram_tensor("acc1T", [128, E_PAD], BF, kind="ExternalOutput")
    acc2T = nc.dram_tensor("acc2T", [128, E_PAD], BF, kind="ExternalOutput")

    SILU = mybir.ActivationFunctionType.Silu
    COPY = mybir.ActivationFunctionType.Copy
    MUL = mybir.AluOpType.mult
    ADD = mybir.AluOpType.add
    EQ = mybir.AluOpType.is_equal
    rg = [list(range(n_cores))]

    gps = nc.gpsimd if GPS_ELEMWISE else nc.vector

    with tile.TileContext(nc) as tc:
        with tc.tile_pool(name="const", bufs=1) as cpool, \
             tc.tile_pool(name="dram", bufs=1, space="DRAM") as dram, \
             tc.tile_pool(name="qpool", bufs=1) as qpool:

            # ---- constants in SBUF ----
            def cload(h, shape, dt=BF):
                t = cpool.tile(shape, dt, tag=h.name)
                nc.sync.dma_start(out=t[:], in_=h[:])
                return t

            iota_tile = cload(iota_tile_h, [128, 512])
            eye = cload(eye_h, [128, 128])
            redmat = cload(redmat_h, [128, 64])
            redmat_wa = cload(redmat_wa_h, [128, 64])
            redmat_wb = cload(redmat_wb_h, [128, 64])
            w2col = cload(w2col_h, [128, 1], F32)
            w3col = cload(w3col_h, [128, 1], F32)
            wproj = cload(wproj_h, [64, 256])
            basis_sb = cload(basis_pm, [128, NT, 4])
            idxcol = cload(idxcol_h, [128, NT])
            l0w1se = cload(l0w1se_h, [64, 256])
            l0w1b = cload(l0w1b_h, [64, 256])
            l0w2a = cload(l0w2a_h, [128, 256])
            l0w2b = cload(l0w2b_h, [128, 256])
            l0w3a = cload(l0w3a_h, [128, 256])
            l0w3b = cload(l0w3b_h, [128, 256])
            l1w1se = cload(l1w1se_h, [64, 256])
            l1w1b = cload(l1w1b_h, [128, 256])
            l1w1c = cload(l1w1c_h, [64, 256])
            l1w2a = cload(l1w2a_h, [128, 256])
            l1w2b = cload(l1w2b_h, [128, 256])
            l1w3a = cload(l1w3a_h, [128, 128])
            l1w3b = cload(l1w3b_h, [128, 128])

            # persistent q = [o0(64)|ov0] , [ov1|ov2] feature-major, bf16
            q0 = qpool.tile([128, E_PAD], BF, tag="q0")
            q1 = qpool.tile([128, E_PAD], BF, tag="q1")

            # DRAM bounce buffers for the chunked AllReduces
            CA = CB * 128                       # atoms per chunk
            t1loc = dram.tile([A, 256], BF)
            t2loc = dram.tile([A, 256], BF)
            t1glob = [dram.tile([CA, 256], BF, addr_space="Shared",
                                name=f"t1g{k}") for k in range(N_CHUNK)]
            t2glob = [dram.tile([CA, 256], BF, addr_space="Shared",
                                name=f"t2g{k}") for k in range(N_CHUNK)]

            def build_M(pool, g, tagsuf=""):
                """One-hot scatter matrix for group g: [128e, 4t, 128a].
                M[p, t, a] = (idx(edge t*128+p) == a), from the resident
                per-tile index column table (free-dim broadcast AP)."""
                m = pool.tile([128, GRP, 128], BF, tag="M" + tagsuf)
                ic = idxcol[:, g * GRP:(g + 1) * GRP]
                icb = bass.AP(ic.tensor, ic.offset,
                              [list(d) for d in ic.ap[:2]] + [[0, 128]])
                itv = iota_tile[:].rearrange("p (t a) -> p t a", t=GRP)
                nc.vector.tensor_tensor(out=m[:], in0=icb, in1=itv, op=EQ)
                return m

            def build_M2(pool, ppool, m, copy_eng):
                """Gather one-hots = transpose of M: [128a, 4t*128e]."""
                pm2 = ppool.tile([128, GRP * 128], BF, tag="pt")
                for t in range(GRP):
                    nc.tensor.transpose(out=pm2[:, t * 128:(t + 1) * 128],
                                        in_=m[:, t, :], identity=eye[:])
                m2 = pool.tile([128, GRP * 128], BF, tag="M2")
                copy_eng.tensor_copy(out=m2[:], in_=pm2[:])
                return m2

            def build_X(ppool, spool, envw_sb, g):
                """Transpose env_w [128f,512e] to edge-major and expand with
                basis -> X [128e, 4t, 256f]."""
                ps = ppool.tile([128, GRP * 128], BF, tag="pt")
                for t in range(GRP):
                    nc.tensor.transpose(
                        out=ps[:, t * 128:(t + 1) * 128],
                        in_=envw_sb[:, t * 128:(t + 1) * 128],
                        identity=eye[:])
                x = spool.tile([128, GRP, 256], BF, tag="X")
                bt = basis_sb[:, g * GRP:(g + 1) * GRP, :]
                psv = ps[:].rearrange("p (t f) -> p t f", t=GRP)   # [128,4,128]
                # X[:, :, 0:64] = envwT[:, :, 0:64] * b0
                pa = psv[:, :, 0:64]
                ba0 = bt[:, :, 0:1]
                ba = bass.AP(ba0.tensor, ba0.offset,
                             [list(d) for d in ba0.ap[:2]] + [[0, 64]])
                nc.vector.tensor_tensor(out=x[:, :, 0:64], in0=pa, in1=ba, op=MUL)
                # X[:, :, 64:256] = envwT[:, :, 64:128] (x3) * b123
                pb0 = psv[:, :, 64:128]
                pb = bass.AP(pb0.tensor, pb0.offset,
                             [list(pb0.ap[0]), list(pb0.ap[1]), [0, 3],
                              list(pb0.ap[2])])
                bb0 = bt[:, :, 1:4]
                bb = bass.AP(bb0.tensor, bb0.offset,
                             [list(d) for d in bb0.ap[:3]] + [[0, 64]])
                xb = x[:, :, 64:256].rearrange("p t (k f) -> p t k f", k=3)
                nc.vector.tensor_tensor(out=xb, in0=pb, in1=bb, op=MUL)
                return x

            # scatter state shared across groups within a phase
            def make_scatter(pool, spool, tloc):
                state = {"slab": None}

                def scatter_tile(t, m_tile, x_tile, sub):
                    b = t // t_b
                    first = (t % t_b == 0)
                    last = (t % t_b == t_b - 1)
                    if first:
                        state["slab"] = pool.tile([128, 256], F32, tag="slab",
                                                  name=f"slab_{t}")
                    nc.tensor.matmul(out=state["slab"][:],
                                     lhsT=m_tile[:, sub, :],
                                     rhs=x_tile[:, sub, :],
                                     start=first, stop=last)
                    if last:
                        sl2 = spool.tile([128, 256], BF, tag="slabsb")
                        nc.vector.tensor_copy(out=sl2[:], in_=state["slab"][:])
                        nc.sync.dma_start(out=tloc[b * 128:(b + 1) * 128, :],
                                          in_=sl2[:])
                return scatter_tile

            # =================== Phase 1: proj + scatter-1 ===================
            with tc.tile_pool(name="p1sb", bufs=3) as sp, \
                 tc.tile_pool(name="p1ps", bufs=2, space="PSUM") as pp, \
                 tc.tile_pool(name="p1pt", bufs=2, space="PSUM") as ppt, \
                 tc.tile_pool(name="p1slab", bufs=2, space="PSUM") as slabp:
                scat1 = make_scatter(slabp, sp, t1loc)
                for g in range(NG):
                    sl = slice(g * GW, (g + 1) * GW)
                    se = sp.tile([64, GW], BF, tag="se")
                    nc.sync.dma_start(out=se[:], in_=seT[:, sl])
                    pj = pp.tile([128, 2, GW], F32, tag="mm")
                    nc.tensor.matmul(out=pj[:, 0, :], lhsT=wproj[:, 0:128],
                                     rhs=se[:], start=True, stop=True)
                    nc.tensor.matmul(out=pj[:, 1, :], lhsT=wproj[:, 128:256],
                                     rhs=se[:], start=True, stop=True)
                    a0ew = sp.tile([128, 2, GW], BF, tag="a0ew")
                    nc.scalar.activation(out=a0ew[:], in_=pj[:], func=COPY)
                    nc.scalar.dma_start(out=acc0T[:, sl], in_=a0ew[:, 0, :])
                    m = build_M(sp, g)
                    x = build_X(ppt, sp, a0ew[:, 1, :], g)
                    for s in range(GRP):
                        scat1(g * GRP + s, m, x, s)

            # chunked collective 1 + per-chunk table loads
            T1c = []
            for k in range(N_CHUNK):
                nc.gpsimd.collective_compute(
                    "AllReduce", ADD, replica_groups=rg,
                    ins=[t1loc[k * CA:(k + 1) * CA, :].opt()],
                    outs=[t1glob[k].opt()])
                tt = qpool.tile([128, CB, 256], BF, tag=f"T1c{k}")
                nc.sync.dma_start(
                    out=tt[:],
                    in_=t1glob[k][:].rearrange("(b p) f -> p b f", p=128))
                T1c.append(tt)

            # =================== Phase 3: gather-1, TP0, mlp0, scatter-2 =====
            with tc.tile_pool(name="p3sb", bufs=2) as sp, \
                 tc.tile_pool(name="p3q", bufs=3) as qp, \
                 tc.tile_pool(name="p3ps", bufs=1, space="PSUM") as pp, \
                 tc.tile_pool(name="p3red", bufs=1, space="PSUM") as redp, \
                 tc.tile_pool(name="p3pt", bufs=1, space="PSUM") as ppt, \
                 tc.tile_pool(name="p3env", bufs=1, space="PSUM") as ep, \
                 tc.tile_pool(name="p3slab", bufs=2, space="PSUM") as slabp:
                scat2 = make_scatter(slabp, sp, t2loc)
                for g in range(NG):
                    sl = slice(g * GW, (g + 1) * GW)
                    m = build_M(sp, g)
                    m2 = build_M2(sp, ppt, m, nc.vector)
                    pe = ep.tile([128, 2, GW], F32, tag="env")
                    t = 0
                    while t < GRP:
                        b = (g * GRP + t) // t_b
                        n = 1
                        while t + n < GRP and (g * GRP + t + n) // t_b == b:
                            n += 1
                        ts = slice(t * 128, (t + n) * 128)
                        Tt = T1c[b // CB]
                        bb = b % CB
                        nc.tensor.matmul(out=pe[:, 0, ts], lhsT=Tt[:, bb, 0:128],
                                         rhs=m2[:, ts], start=True, stop=True)
                        nc.tensor.matmul(out=pe[:, 1, ts], lhsT=Tt[:, bb, 128:256],
                                         rhs=m2[:, ts], start=True, stop=True)
                        t += n
                    e = sp.tile([128, 2, GW], BF, tag="e")
                    nc.scalar.activation(out=e[:], in_=pe[:], func=COPY)
                    e0 = e[:, 0, :]
                    e1 = e[:, 1, :]
                    se3 = sp.tile([64, GW], BF, tag="se3")
                    nc.sync.dma_start(out=se3[:], in_=seT[:, sl])
                    tf2 = sp.tile([128, 2, GW], BF, tag="tf2")
                    nc.sync.dma_start(out=tf2[:],
                                      in_=tfT[:, :, sl].rearrange("c p e -> p c e"))
                    tf0 = tf2[:, 0, :]
                    tf1 = tf2[:, 1, :]
                    # ---- TP0 ----
                    # o0 = redmat_wa^T (tf0*e0) + redmat_wb^T (tf1*e1)
                    ma = sp.tile([128, GW], BF, tag="ma")
                    mb = sp.tile([128, GW], BF, tag="mb")
                    nc.vector.tensor_tensor(out=ma[:], in0=tf0, in1=e0, op=MUL)
                    nc.vector.tensor_tensor(out=mb[:], in0=tf1, in1=e1, op=MUL)
                    po0 = redp.tile([64, GW], F32, tag="po0")
                    nc.tensor.matmul(out=po0[:], lhsT=redmat_wa[:], rhs=ma[:],
                                     start=True, stop=False)
                    nc.tensor.matmul(out=po0[:], lhsT=redmat_wb[:], rhs=mb[:],
                                     start=False, stop=True)
                    nc.vector.tensor_copy(out=q0[0:64, sl], in_=po0[:])
                    # hi = [sa; sa], t0rep = [t0; t0]
                    hi = qp.tile([128, GW], BF, tag="hi")
                    nc.vector.tensor_copy(out=hi[0:64, :], in_=e[0:64, 0, :])
                    nc.vector.tensor_copy(out=hi[64:128, :], in_=e[0:64, 0, :])
                    # t0w = [w3*t0; w3*t0] (w3 folded so GPS can use plain TT)
                    t0w = qp.tile([128, GW], BF, tag="t0w")
                    nc.vector.tensor_scalar_mul(out=t0w[0:64, :],
                                                in0=tf2[0:64, 0, :],
                                                scalar1=w3col[0:64, :])
                    nc.vector.tensor_scalar_mul(out=t0w[64:128, :],
                                                in0=tf2[0:64, 0, :],
                                                scalar1=w3col[0:64, :])
                    # ov_k = w2*sa*tv_k + w3*t0*va_k
                    pa0 = qp.tile([128, GW], BF, tag="pa0")
                    pa1 = qp.tile([128, GW], BF, tag="pa1")
                    pb0 = qp.tile([128, GW], BF, tag="pb0")
                    pb1 = qp.tile([128, GW], BF, tag="pb1")
                    nc.vector.scalar_tensor_tensor(
                        out=pa0[:], in0=tf0, scalar=w2col[:], in1=hi[:],
                        op0=MUL, op1=MUL)
                    nc.vector.scalar_tensor_tensor(
                        out=pa1[:], in0=tf1, scalar=w2col[:], in1=hi[:],
                        op0=MUL, op1=MUL)
                    gps.tensor_tensor(out=pb0[:], in0=t0w[:], in1=e0, op=MUL)
                    gps.tensor_tensor(out=pb1[:], in0=t0w[:], in1=e1, op=MUL)
                    gps.tensor_tensor(out=q0[64:128, sl], in0=pa0[64:128, :],
                                      in1=pb0[64:128, :], op=ADD)
                    gps.tensor_tensor(out=q1[:, sl], in0=pa1[:], in1=pb1[:],
                                      op=ADD)
                    # ---- mlp0: x = [se-folded ; o0] ----
                    ph1 = pp.tile([128, 2, GW], F32, tag="mm")
                    for mbi in range(2):
                        msl = slice(mbi * 128, (mbi + 1) * 128)
                        nc.tensor.matmul(out=ph1[:, mbi, :], lhsT=l0w1se[:, msl],
                                         rhs=se3[:], start=True, stop=False)
                        nc.tensor.matmul(out=ph1[:, mbi, :], lhsT=l0w1b[:, msl],
                                         rhs=q0[0:64, sl], start=False, stop=True)
                    h1 = sp.tile([128, 2, GW], BF, tag="h1")
                    nc.scalar.activation(out=h1[:], in_=ph1[:], func=SILU)
                    ph2 = pp.tile([128, 2, GW], F32, tag="mm")
                    for mbi in range(2):
                        msl = slice(mbi * 128, (mbi + 1) * 128)
                        nc.tensor.matmul(out=ph2[:, mbi, :], lhsT=l0w2a[:, msl],
                                         rhs=h1[:, 0, :], start=True, stop=False)
                        nc.tensor.matmul(out=ph2[:, mbi, :], lhsT=l0w2b[:, msl],
                                         rhs=h1[:, 1, :], start=False, stop=True)
                    h2 = sp.tile([128, 2, GW], BF, tag="h2")
                    nc.scalar.activation(out=h2[:], in_=ph2[:], func=SILU)
                    # lat = h2 @ W3 : slot0 = acc1, slot1 = env_w2
                    pw3 = pp.tile([128, 2, GW], F32, tag="mm")
                    for mbi in range(2):
                        msl = slice(mbi * 128, (mbi + 1) * 128)
                        nc.tensor.matmul(out=pw3[:, mbi, :], lhsT=l0w3a[:, msl],
                                         rhs=h2[:, 0, :], start=True, stop=False)
                        nc.tensor.matmul(out=pw3[:, mbi, :], lhsT=l0w3b[:, msl],
                                         rhs=h2[:, 1, :], start=False, stop=True)
                    law = sp.tile([128, 2, GW], BF, tag="law")
                    nc.vector.tensor_copy(out=law[:], in_=pw3[:])
                    nc.sync.dma_start(out=acc1T[:, sl], in_=law[:, 0, :])
                    x2 = build_X(ppt, sp, law[:, 1, :], g)
                    for s in range(GRP):
                        scat2(g * GRP + s, m, x2, s)

            # chunked collective 2 + per-chunk table loads
            T2c = []
            for k in range(N_CHUNK):
                nc.gpsimd.collective_compute(
                    "AllReduce", ADD, replica_groups=rg,
                    ins=[t2loc[k * CA:(k + 1) * CA, :].opt()],
                    outs=[t2glob[k].opt()])
                tt = qpool.tile([128, CB, 256], BF, tag=f"T2c{k}")
                nc.sync.dma_start(
                    out=tt[:],
                    in_=t2glob[k][:].rearrange("(b p) f -> p b f", p=128))
                T2c.append(tt)

            # =================== Phase 5: gather-2, TP1, mlp1 ================
            with tc.tile_pool(name="p5sb", bufs=2) as sp, \
                 tc.tile_pool(name="p5ps", bufs=1, space="PSUM") as pp, \
                 tc.tile_pool(name="p5out", bufs=2, space="PSUM") as outp, \
                 tc.tile_pool(name="p5red", bufs=1, space="PSUM") as redp, \
                 tc.tile_pool(name="p5pt", bufs=1, space="PSUM") as ppt, \
                 tc.tile_pool(name="p5env", bufs=1, space="PSUM") as ep:
                for g in range(NG):
                    sl = slice(g * GW, (g + 1) * GW)
                    m = build_M(sp, g)
                    m2 = build_M2(sp, ppt, m, nc.vector)
                    pe = ep.tile([128, 2, GW], F32, tag="env")
                    t = 0
                    while t < GRP:
                        b = (g * GRP + t) // t_b
                        n = 1
                        while t + n < GRP and (g * GRP + t + n) // t_b == b:
                            n += 1
                        ts = slice(t * 128, (t + n) * 128)
                        Tt = T2c[b // CB]
                        bb = b % CB
                        nc.tensor.matmul(out=pe[:, 0, ts], lhsT=Tt[:, bb, 0:128],
                                         rhs=m2[:, ts], start=True, stop=True)
                        nc.tensor.matmul(out=pe[:, 1, ts], lhsT=Tt[:, bb, 128:256],
                                         rhs=m2[:, ts], start=True, stop=True)
                        t += n
                    e = sp.tile([128, 2, GW], BF, tag="e")
                    nc.scalar.activation(out=e[:], in_=pe[:], func=COPY)
                    # o_last = redmat^T (e0*q0) + redmat^T (e1*q1)
                    pa = sp.tile([128, GW], BF, tag="pa")
                    pb = sp.tile([128, GW], BF, tag="pb")
                    nc.vector.tensor_tensor(out=pa[:], in0=e[:, 0, :],
                                            in1=q0[:, sl], op=MUL)
                    nc.vector.tensor_tensor(out=pb[:], in0=e[:, 1, :],
                                            in1=q1[:, sl], op=MUL)
                    pol = redp.tile([64, GW], F32, tag="pol")
                    nc.tensor.matmul(out=pol[:], lhsT=redmat[:], rhs=pa[:],
                                     start=True, stop=False)
                    nc.tensor.matmul(out=pol[:], lhsT=redmat[:], rhs=pb[:],
                                     start=False, stop=True)
                    ol = sp.tile([64, GW], BF, tag="ol")
                    nc.vector.tensor_copy(out=ol[:], in_=pol[:])
                    # mlp1: x = [se-folded ; acc1 ; o_last]
                    se5 = sp.tile([64, GW], BF, tag="se5")
                    nc.sync.dma_start(out=se5[:], in_=seT[:, sl])
                    ac1 = sp.tile([128, GW], BF, tag="ac1")
                    nc.sync.dma_start(out=ac1[:], in_=acc1T[:, sl])
                    ph1t = pp.tile([128, 2, GW], F32, tag="mm")
                    for mbi in range(2):
                        msl = slice(mbi * 128, (mbi + 1) * 128)
                        nc.tensor.matmul(out=ph1t[:, mbi, :], lhsT=l1w1se[:, msl],
                                         rhs=se5[:], start=True, stop=False)
                        nc.tensor.matmul(out=ph1t[:, mbi, :], lhsT=l1w1b[:, msl],
                                         rhs=ac1[:], start=False, stop=False)
                        nc.tensor.matmul(out=ph1t[:, mbi, :], lhsT=l1w1c[:, msl],
                                         rhs=ol[:], start=False, stop=True)
                    h1 = sp.tile([128, 2, GW], BF, tag="h1")
                    nc.scalar.activation(out=h1[:], in_=ph1t[:], func=SILU)
                    ph2t = pp.tile([128, 2, GW], F32, tag="mm")
                    for mbi in range(2):
                        msl = slice(mbi * 128, (mbi + 1) * 128)
                        nc.tensor.matmul(out=ph2t[:, mbi, :], lhsT=l1w2a[:, msl],
                                         rhs=h1[:, 0, :], start=True, stop=False)
                        nc.tensor.matmul(out=ph2t[:, mbi, :], lhsT=l1w2b[:, msl],
                                         rhs=h1[:, 1, :], start=False, stop=True)
                    h2 = sp.tile([128, 2, GW], BF, tag="h2")
                    nc.scalar.activation(out=h2[:], in_=ph2t[:], func=SILU)
                    pw3 = outp.tile([128, GW], F32, tag="out")
                    nc.tensor.matmul(out=pw3[:], lhsT=l1w3a[:], rhs=h2[:, 0, :],
                                     start=True, stop=False)
                    nc.tensor.matmul(out=pw3[:], lhsT=l1w3b[:], rhs=h2[:, 1, :],
                                     start=False, stop=True)
                    a2sb = sp.tile([128, GW], BF, tag="a2sb")
                    nc.vector.tensor_copy(out=a2sb[:], in_=pw3[:])
                    nc.sync.dma_start(out=acc2T[:, sl], in_=a2sb[:])

    nc.compile()
    return nc, dict(NT=NT, E_PAD=E_PAD, A=A, n_blk=n_blk, t_b=t_b)


# =====================================================================
# Host side
# =====================================================================

def _prep_core(centers_l, se_l, tf_l, basis_l, n_blk, t_b):
    """Sort/pad one core's edges into the block-tile layout."""
    NT = n_blk * t_b
    E_PAD = NT * 128
    order = np.argsort(centers_l, kind="stable")
    bid = centers_l[order] // ABLK
    perm = np.full(E_PAD, -1, np.int64)
    for b in range(n_blk):
        run = order[bid == b]
        assert len(run) <= t_b * 128, f"block {b} overflow: {len(run)}"
        perm[b * t_b * 128: b * t_b * 128 + len(run)] = run
    valid = perm >= 0
    psafe = np.where(valid, perm, 0)

    idxrel = np.where(
        valid, centers_l[psafe] - ABLK * (np.arange(E_PAD) // (t_b * 128)),
        -1).astype(np.float32)

    se_pad = se_l[psafe] * valid[:, None]          # [E_PAD, 64]
    tf_pad = tf_l[psafe] * valid[:, None, None]    # [E_PAD, 64, 4]
    basis_pad = basis_l[psafe] * valid[:, None]    # [E_PAD, 4]

    seT = np.ascontiguousarray(se_pad.T).astype(BF_NP)
    # raw tensor features, feature-major: rows f = 64*comp+chan, two chunks
    tf_cm = np.ascontiguousarray(
        tf_pad.transpose(2, 1, 0)).reshape(4 * 64, E_PAD).astype(np.float32)
    tfT = np.ascontiguousarray(
        np.stack([tf_cm[0:128], tf_cm[128:256]])).astype(BF_NP)
    basis_pm = np.ascontiguousarray(
        basis_pad.reshape(NT, 128, 4).transpose(1, 0, 2)).astype(BF_NP)
    idxcol = np.ascontiguousarray(
        idxrel.reshape(NT, 128).T).astype(BF_NP)           # [128, NT]
    return dict(seT=seT, tfT=tfT, basis_pm=basis_pm, idxcol=idxcol), perm


def _weights_maps(W_proj, w_tp0, w_tp1, l0, l1):
    bf = lambda a: np.ascontiguousarray(a).astype(BF_NP)
    f32 = np.float32
    w0 = (w_tp0[0] * NORM).astype(f32)
    w1 = (w_tp0[1] * NORM * INV_SQRT3).astype(f32)
    w2 = (w_tp0[2] * NORM).astype(f32)
    w3 = (w_tp0[3] * NORM).astype(f32)
    wx = np.concatenate([w_tp1[0] * NORM,
                         w_tp1[1] * NORM * INV_SQRT3]).astype(f32)  # [128]
    eye64 = np.eye(64, dtype=f32)
    redmat = np.tile(eye64, (2, 1))
    redmat_wa = np.concatenate([eye64 * w0, eye64 * w1], 0)   # [128, 64]
    redmat_wb = np.concatenate([eye64 * w1, eye64 * w1], 0)
    l0w1, l0w2, l0w3 = l0
    l0w3 = l0w3.copy()
    l0w3[:, 128:256] = l0w3[:, 128:256] * wx[None, :]
    l1w1, l1w2, l1w3 = l1
    Wp = np.asarray(W_proj, f32)
    l0w1se = Wp[:, 0:128] @ l0w1[0:128]       # [64, 256]
    l1w1se = Wp[:, 0:128] @ l1w1[0:128]       # [64, 256]
    return {
        "iota_tile": bf(np.tile(np.arange(128, dtype=f32)[None, :], (128, 4))),
        "eye": bf(np.eye(128, dtype=f32)),
        "redmat": bf(redmat),
        "redmat_wa": bf(redmat_wa),
        "redmat_wb": bf(redmat_wb),
        "w2col": np.ascontiguousarray(np.tile(w2, 2)[:, None], f32),
        "w3col": np.ascontiguousarray(np.tile(w3, 2)[:, None], f32),
        "wproj": bf(Wp),
        "l0w1se": bf(l0w1se), "l0w1b": bf(l0w1[128:192]),
        "l0w2a": bf(l0w2[0:128]), "l0w2b": bf(l0w2[128:256]),
        "l0w3a": bf(l0w3[0:128]), "l0w3b": bf(l0w3[128:256]),
        "l1w1se": bf(l1w1se), "l1w1b": bf(l1w1[128:256]),
        "l1w1c": bf(l1w1[256:320]),
        "l1w2a": bf(l1w2[0:128]), "l1w2b": bf(l1w2[128:256]),
        "l1w3a": bf(l1w3[0:128]), "l1w3b": bf(l1w3[128:256]),
    }


_CACHE = {}


def kernel(edge_index, num_atoms, tensor_basis, tensor_features, scalar_embed,
           W_proj, w_tp0, w_tp1,
           lat0_W1, lat0_W2, lat0_W3, lat1_W1, lat1_W2, lat1_W3,
           _trace=False, _tmpdir=None):
    if "nc" not in _CACHE:
        _CACHE["nc"], _CACHE["meta"] = build_graph()
    nc, meta = _CACHE["nc"], _CACHE["meta"]
    n_blk, t_b, E_PAD = meta["n_blk"], meta["t_b"], meta["E_PAD"]

    edge_index = np.asarray(edge_index)
    centers = edge_index[0]
    tb = np.asarray(tensor_basis, np.float32)
    tf = np.asarray(tensor_features, np.float32)
    se = np.asarray(scalar_embed, np.float32)

    wmaps = _weights_maps(
        np.asarray(W_proj, np.float32), np.asarray(w_tp0, np.float32),
        np.asarray(w_tp1, np.float32),
        (np.asarray(lat0_W1, np.float32), np.asarray(lat0_W2, np.float32),
         np.asarray(lat0_W3, np.float32)),
        (np.asarray(lat1_W1, np.float32), np.asarray(lat1_W2, np.float32),
         np.asarray(lat1_W3, np.float32)))

    in_maps, perms = [], []
    for c in range(N_CORES):
        s = slice(c * E_LOC, (c + 1) * E_LOC)
        m, perm = _prep_core(centers[s], se[s], tf[s], tb[s], n_blk, t_b)
        m.update(wmaps)
        in_maps.append(m)
        perms.append(perm)

    res = run_bass_kernel_spmd(nc, in_maps, core_ids=list(range(N_CORES)),
                               trace=_trace, tmpdir=_tmpdir)
    out = np.empty((E, NS * 3), np.float32)
    for c in range(N_CORES):
        r = res.results[c]
        op = np.concatenate(
            [np.asarray(r["acc0T"]).astype(np.float32).T,
             np.asarray(r["acc1T"]).astype(np.float32).T,
             np.asarray(r["acc2T"]).astype(np.float32).T], axis=1)  # [E_PAD,384]
        perm = perms[c]
        valid = perm >= 0
        out[c * E_LOC + perm[valid]] = op[valid]
    if _trace:
        kernel.last_exec_time_ns = res.exec_time_ns
        if res.instructions_and_trace:
            kernel.last_trace = res.instructions_and_trace[1]
    return out


# revision 28
# speedup vs baseline: 1.4017x; 1.4017x over previous
"""Allegro-style GNN message passing on 8 TRN2 NeuronCores.

Strategy (edges sharded contiguously across 8 cores):
  - Per core 20000 edges, sorted by edge_center, bucketed into 40 blocks of
    128 atoms, each block padded to T_B=5 tiles of 128 edges (E_pad=25600).
  - Feature-major (transposed) activation layout [feat<=128, edges].
  - segment_sum  = per-tile one-hot matmul (PE) accumulating per-block PSUM
    slabs -> bf16 local table [5120, 256] -> chunked AllReduce (bf16).
  - gather       = per-tile one-hot matmul against the AllReduced table slab.
  - One-hot matrices built on-chip from a tiny index table (no index DMA).
  - W_proj folded into the latent-MLP first layers on host (acc0 never
    re-read on device); tensor features loaded raw once, TP weights applied
    on-chip via scalar_tensor_tensor / weighted reduction matrices.
  - MLPs in bf16 on the TensorEngine with fp32 PSUM accumulation.
Host does layout prep (sort/pad/transpose/cast) + unshard only.
"""

import sys
import math

sys.path.insert(0, "/opt/trn_rl_repo")

import numpy as np
import ml_dtypes

import concourse.bass as bass
import concourse.bacc as bacc
from concourse import tile
import concourse.mybir as mybir
from concourse.bass_utils import run_bass_kernel_spmd

BF = mybir.dt.bfloat16
F32 = mybir.dt.float32
BF_NP = ml_dtypes.bfloat16

# problem constants
E = 160000
N_ATOMS = 5000
C = 64
NS = 128
SCAL_IN = 64
HID = 256
NORM = 1.0 / math.sqrt(32.0)
INV_SQRT3 = 1.0 / math.sqrt(3.0)

N_CORES = 8
E_LOC = E // N_CORES          # 20000
ABLK = 128                    # atoms per block
T_B = 5                       # tiles (of 128 edges) per block
GRP = 4                       # tiles per matmul group (N=512)
N_CHUNK = 4                   # AllReduce chunks

# route the off-critical ov-chain elementwise ops to GPSIMD (Pool engine)
GPS_ELEMWISE = True


def build_graph(n_cores=N_CORES, n_blk=40, t_b=T_B):
    """Build the SPMD Bass graph (same graph runs on every core)."""
    NT = n_blk * t_b              # edge tiles
    E_PAD = NT * 128
    A = n_blk * ABLK              # padded atom count
    assert NT % GRP == 0
    NG = NT // GRP
    GW = GRP * 128
    assert n_blk % N_CHUNK == 0
    CB = n_blk // N_CHUNK         # blocks per AR chunk

    nc = bacc.Bacc("TRN2", target_bir_lowering=False, debug=False,
                   num_devices=n_cores)

    # ---- I/O ----
    seT = nc.dram_tensor("seT", [SCAL_IN, E_PAD], BF, kind="ExternalInput")
    tfT = nc.dram_tensor("tfT", [2, 128, E_PAD], BF, kind="ExternalInput")
    t0wT = nc.dram_tensor("t0wT", [128, E_PAD], BF, kind="ExternalInput")
    idxbc_h = nc.dram_tensor("idxbc", [128, E_PAD], BF, kind="ExternalInput")
    basis_pm = nc.dram_tensor("basis_pm", [128, NT, 4], BF, kind="ExternalInput")
    idxcol_h = nc.dram_tensor("idxcol", [128, NT], BF, kind="ExternalInput")
    iota_part_h = nc.dram_tensor("iota_part", [128, 512], BF, kind="ExternalInput")
    iota_tile_h = nc.dram_tensor("iota_tile", [128, 512], BF, kind="ExternalInput")
    eye_h = nc.dram_tensor("eye", [128, 128], BF, kind="ExternalInput")
    redmat_h = nc.dram_tensor("redmat", [128, 64], BF, kind="ExternalInput")
    redmat_wa_h = nc.dram_tensor("redmat_wa", [128, 64], BF, kind="ExternalInput")
    redmat_wb_h = nc.dram_tensor("redmat_wb", [128, 64], BF, kind="ExternalInput")
    w2col_h = nc.dram_tensor("w2col", [128, 1], F32, kind="ExternalInput")
    w3col_h = nc.dram_tensor("w3col", [128, 1], F32, kind="ExternalInput")
    wproj_h = nc.dram_tensor("wproj", [64, 256], BF, kind="ExternalInput")
    # lat0: W1se [64,256] (W_proj folded), W1b [64,256]; W2/W3 [256,256]
    l0w1se_h = nc.dram_tensor("l0w1se", [64, 256], BF, kind="ExternalInput")
    l0w1b_h = nc.dram_tensor("l0w1b", [64, 256], BF, kind="ExternalInput")
    l0w2a_h = nc.dram_tensor("l0w2a", [128, 256], BF, kind="ExternalInput")
    l0w2b_h = nc.dram_tensor("l0w2b", [128, 256], BF, kind="ExternalInput")
    l0w3a_h = nc.dram_tensor("l0w3a", [128, 256], BF, kind="ExternalInput")
    l0w3b_h = nc.dram_tensor("l0w3b", [128, 256], BF, kind="ExternalInput")
    # lat1: W1se [64,256], W1b [128,256] (acc1), W1c [64,256] (ol)
    l1w1se_h = nc.dram_tensor("l1w1se", [64, 256], BF, kind="ExternalInput")
    l1w1b_h = nc.dram_tensor("l1w1b", [128, 256], BF, kind="ExternalInput")
    l1w1c_h = nc.dram_tensor("l1w1c", [64, 256], BF, kind="ExternalInput")
    l1w2a_h = nc.dram_tensor("l1w2a", [128, 256], BF, kind="ExternalInput")
    l1w2b_h = nc.dram_tensor("l1w2b", [128, 256], BF, kind="ExternalInput")
    l1w3a_h = nc.dram_tensor("l1w3a", [128, 128], BF, kind="ExternalInput")
    l1w3b_h = nc.dram_tensor("l1w3b", [128, 128], BF, kind="ExternalInput")

    acc0T = nc.dram_tensor("acc0T", [128, E_PAD], BF, kind="ExternalOutput")
    acc1T = nc.dram_tensor("acc1T", [128, E_PAD], BF, kind="ExternalOutput")
    acc2T = nc.dram_tensor("acc2T", [128, E_PAD], BF, kind="ExternalOutput")

    SILU = mybir.ActivationFunctionType.Silu
    COPY = mybir.ActivationFunctionType.Copy
    MUL = mybir.AluOpType.mult
    ADD = mybir.AluOpType.add
    EQ = mybir.AluOpType.is_equal
    rg = [list(range(n_cores))]

    gps = nc.gpsimd if GPS_ELEMWISE else nc.vector

    with tile.TileContext(nc) as tc:
        with tc.tile_pool(name="const", bufs=1) as cpool, \
             tc.tile_pool(name="dram", bufs=1, space="DRAM") as dram, \
             tc.tile_pool(name="qpool", bufs=1) as qpool:

            # ---- constants in SBUF ----
            def cload(h, shape, dt=BF):
                t = cpool.tile(shape, dt, tag=h.name)
                nc.sync.dma_start(out=t[:], in_=h[:])
                return t

            iota_part = cload(iota_part_h, [128, 512])
            iota_tile = cload(iota_tile_h, [128, 512])
            eye = cload(eye_h, [128, 128])
            redmat = cload(redmat_h, [128, 64])
            redmat_wa = cload(redmat_wa_h, [128, 64])
            redmat_wb = cload(redmat_wb_h, [128, 64])
            w2col = cload(w2col_h, [128, 1], F32)
            w3col = cload(w3col_h, [128, 1], F32)
            wproj = cload(wproj_h, [64, 256])
            basis_sb = cload(basis_pm, [128, NT, 4])
            idxcol = cload(idxcol_h, [128, NT])
            l0w1se = cload(l0w1se_h, [64, 256])
            l0w1b = cload(l0w1b_h, [64, 256])
            l0w2a = cload(l0w2a_h, [128, 256])
            l0w2b = cload(l0w2b_h, [128, 256])
            l0w3a = cload(l0w3a_h, [128, 256])
            l0w3b = cload(l0w3b_h, [128, 256])
            l1w1se = cload(l1w1se_h, [64, 256])
            l1w1b = cload(l1w1b_h, [128, 256])
            l1w1c = cload(l1w1c_h, [64, 256])
            l1w2a = cload(l1w2a_h, [128, 256])
            l1w2b = cload(l1w2b_h, [128, 256])
            l1w3a = cload(l1w3a_h, [128, 128])
            l1w3b = cload(l1w3b_h, [128, 128])

            # persistent q = [o0(64)|ov0] , [ov1|ov2] feature-major, bf16
            q0 = qpool.tile([128, E_PAD], BF, tag="q0")
            q1 = qpool.tile([128, E_PAD], BF, tag="q1")

            # DRAM bounce buffers for the chunked AllReduces
            CA = CB * 128                       # atoms per chunk
            t1loc = dram.tile([A, 256], BF)
            t2loc = dram.tile([A, 256], BF)
            t1glob = [dram.tile([CA, 256], BF, addr_space="Shared",
                                name=f"t1g{k}") for k in range(N_CHUNK)]
            t2glob = [dram.tile([CA, 256], BF, addr_space="Shared",
                                name=f"t2g{k}") for k in range(N_CHUNK)]

            def build_M(pool, g, eng=None):
                """One-hot scatter matrix for group g: [128e, 4t, 128a].
                M[p, t, a] = (idx(edge t*128+p) == a), from the resident
                per-tile index column table (free-dim broadcast AP)."""
                m = pool.tile([128, GRP, 128], BF, tag="M")
                ic = idxcol[:, g * GRP:(g + 1) * GRP]
                icb = bass.AP(ic.tensor, ic.offset,
                              [list(d) for d in ic.ap[:2]] + [[0, 128]])
                itv = iota_tile[:].rearrange("p (t a) -> p t a", t=GRP)
                (eng or nc.vector).tensor_tensor(out=m[:], in0=icb, in1=itv, op=EQ)
                return m

            def build_M2(pool, ppool, m, copy_eng):
                """Gather one-hots = transpose of M: [128a, 4t*128e]."""
                pm2 = ppool.tile([128, GRP * 128], BF, tag="ptm")
                for t in range(GRP):
                    nc.tensor.transpose(out=pm2[:, t * 128:(t + 1) * 128],
                                        in_=m[:, t, :], identity=eye[:])
                m2 = pool.tile([128, GRP * 128], BF, tag="M2")
                copy_eng.tensor_copy(out=m2[:], in_=pm2[:])
                return m2

            def build_M2_eq(pool, g):
                """Gather one-hots via EQ against a DMA'd index row
                broadcast: M2[p, e] = (idx(e) == p)."""
                sl = slice(g * GW, (g + 1) * GW)
                ib = pool.tile([128, GW], BF, tag="idxbc")
                nc.sync.dma_start(out=ib[:], in_=idxbc_h[:, sl])
                m2 = pool.tile([128, GW], BF, tag="M2")
                nc.vector.tensor_tensor(out=m2[:], in0=iota_part[:],
                                        in1=ib[:], op=EQ)
                return m2

            def build_X(ppool, spool, envw_sb, g):
                """Transpose env_w [128f,512e] to edge-major and expand with
                basis -> X [128e, 4t, 256f]."""
                ps = ppool.tile([128, GRP * 128], BF, tag="pt")
                for t in range(GRP):
                    nc.tensor.transpose(
                        out=ps[:, t * 128:(t + 1) * 128],
                        in_=envw_sb[:, t * 128:(t + 1) * 128],
                        identity=eye[:])
                x = spool.tile([128, GRP, 256], BF, tag="X")
                bt = basis_sb[:, g * GRP:(g + 1) * GRP, :]
                psv = ps[:].rearrange("p (t f) -> p t f", t=GRP)   # [128,4,128]
                # X[:, :, 0:64] = envwT[:, :, 0:64] * b0
                pa = psv[:, :, 0:64]
                ba0 = bt[:, :, 0:1]
                ba = bass.AP(ba0.tensor, ba0.offset,
                             [list(d) for d in ba0.ap[:2]] + [[0, 64]])
                nc.vector.tensor_tensor(out=x[:, :, 0:64], in0=pa, in1=ba, op=MUL)
                # X[:, :, 64:256] = envwT[:, :, 64:128] (x3) * b123
                pb0 = psv[:, :, 64:128]
                pb = bass.AP(pb0.tensor, pb0.offset,
                             [list(pb0.ap[0]), list(pb0.ap[1]), [0, 3],
                              list(pb0.ap[2])])
                bb0 = bt[:, :, 1:4]
                bb = bass.AP(bb0.tensor, bb0.offset,
                             [list(d) for d in bb0.ap[:3]] + [[0, 64]])
                xb = x[:, :, 64:256].rearrange("p t (k f) -> p t k f", k=3)
                nc.vector.tensor_tensor(out=xb, in0=pb, in1=bb, op=MUL)
                return x

            # scatter state shared across groups within a phase; both
            # in-flight slabs live in ONE 2-slot PSUM tile (bank economy)
            def make_scatter(pool, spool, tloc):
                slabT = pool.tile([128, 2, 256], F32, tag="slab")

                def scatter_tile(t, m_tile, x_tile, sub):
                    b = t // t_b
                    first = (t % t_b == 0)
                    last = (t % t_b == t_b - 1)
                    nc.tensor.matmul(out=slabT[:, b % 2, :],
                                     lhsT=m_tile[:, sub, :],
                                     rhs=x_tile[:, sub, :],
                                     start=first, stop=last)
                    if last:
                        sl2 = spool.tile([128, 256], BF, tag="slabsb")
                        nc.vector.tensor_copy(out=sl2[:], in_=slabT[:, b % 2, :])
                        nc.sync.dma_start(out=tloc[b * 128:(b + 1) * 128, :],
                                          in_=sl2[:])
                return scatter_tile

            # =================== Phase 1: proj + scatter-1 ===================
            with tc.tile_pool(name="p1sb", bufs=3) as sp, \
                 tc.tile_pool(name="p1ps", bufs=2, space="PSUM") as pp, \
                 tc.tile_pool(name="p1pt", bufs=2, space="PSUM") as ppt, \
                 tc.tile_pool(name="p1slab", bufs=1, space="PSUM") as slabp:
                scat1 = make_scatter(slabp, sp, t1loc)
                for g in range(NG):
                    sl = slice(g * GW, (g + 1) * GW)
                    se = sp.tile([64, GW], BF, tag="se")
                    nc.sync.dma_start(out=se[:], in_=seT[:, sl])
                    m = build_M(sp, g)
                    pj = pp.tile([128, 2, GW], F32, tag="mm")
                    nc.tensor.matmul(out=pj[:, 0, :], lhsT=wproj[:, 0:128],
                                     rhs=se[:], start=True, stop=True)
                    nc.tensor.matmul(out=pj[:, 1, :], lhsT=wproj[:, 128:256],
                                     rhs=se[:], start=True, stop=True)
                    a0ew = sp.tile([128, 2, GW], BF, tag="a0ew")
                    nc.scalar.activation(out=a0ew[:], in_=pj[:], func=COPY)
                    nc.scalar.dma_start(out=acc0T[:, sl], in_=a0ew[:, 0, :])
                    x = build_X(ppt, sp, a0ew[:, 1, :], g)
                    for s in range(GRP):
                        scat1(g * GRP + s, m, x, s)

            # chunked collective 1 + per-chunk table loads
            T1c = []
            for k in range(N_CHUNK):
                nc.gpsimd.collective_compute(
                    "AllReduce", ADD, replica_groups=rg,
                    ins=[t1loc[k * CA:(k + 1) * CA, :].opt()],
                    outs=[t1glob[k].opt()])
                tt = qpool.tile([128, CB, 256], BF, tag=f"T1c{k}")
                nc.sync.dma_start(
                    out=tt[:],
                    in_=t1glob[k][:].rearrange("(b p) f -> p b f", p=128))
                T1c.append(tt)

            # =================== Phase 3: gather-1, TP0, mlp0, scatter-2 =====
            with tc.tile_pool(name="p3sb", bufs=2) as sp, \
                 tc.tile_pool(name="p3q", bufs=3) as qp, \
                 tc.tile_pool(name="p3ps", bufs=1, space="PSUM") as pp, \
                 tc.tile_pool(name="p3red", bufs=1, space="PSUM") as redp, \
                 tc.tile_pool(name="p3pt", bufs=1, space="PSUM") as ppt, \
                 tc.tile_pool(name="p3env", bufs=1, space="PSUM") as ep, \
                 tc.tile_pool(name="p3slab", bufs=1, space="PSUM") as slabp:
                scat2 = make_scatter(slabp, sp, t2loc)
                for g in range(NG):
                    sl = slice(g * GW, (g + 1) * GW)
                    se3 = sp.tile([64, GW], BF, tag="se3")
                    nc.sync.dma_start(out=se3[:], in_=seT[:, sl])
                    tf2 = sp.tile([128, 2, GW], BF, tag="tf2")
                    nc.sync.dma_start(out=tf2[:],
                                      in_=tfT[:, :, sl].rearrange("c p e -> p c e"))
                    t0w = sp.tile([128, GW], BF, tag="t0w")
                    nc.sync.dma_start(out=t0w[:], in_=t0wT[:, sl])
                    m = build_M(sp, g)
                    m2 = build_M2(sp, ppt, m, nc.vector)
                    pe = ep.tile([128, 2, GW], F32, tag="env")
                    t = 0
                    while t < GRP:
                        b = (g * GRP + t) // t_b
                        n = 1
                        while t + n < GRP and (g * GRP + t + n) // t_b == b:
                            n += 1
                        ts = slice(t * 128, (t + n) * 128)
                        Tt = T1c[b // CB]
                        bb = b % CB
                        nc.tensor.matmul(out=pe[:, 0, ts], lhsT=Tt[:, bb, 0:128],
                                         rhs=m2[:, ts], start=True, stop=True)
                        nc.tensor.matmul(out=pe[:, 1, ts], lhsT=Tt[:, bb, 128:256],
                                         rhs=m2[:, ts], start=True, stop=True)
                        t += n
                    e = sp.tile([128, 2, GW], BF, tag="e")
                    nc.scalar.activation(out=e[:], in_=pe[:], func=COPY)
                    e0 = e[:, 0, :]
                    e1 = e[:, 1, :]
                    tf0 = tf2[:, 0, :]
                    tf1 = tf2[:, 1, :]
                    # ---- TP0 ----
                    # o0 = redmat_wa^T (tf0*e0) + redmat_wb^T (tf1*e1)
                    ma = sp.tile([128, GW], BF, tag="ma")
                    mb = sp.tile([128, GW], BF, tag="mb")
                    nc.vector.tensor_tensor(out=ma[:], in0=tf0, in1=e0, op=MUL)
                    nc.vector.tensor_tensor(out=mb[:], in0=tf1, in1=e1, op=MUL)
                    po0 = redp.tile([64, GW], F32, tag="po0")
                    nc.tensor.matmul(out=po0[:], lhsT=redmat_wa[:], rhs=ma[:],
                                     start=True, stop=False)
                    nc.tensor.matmul(out=po0[:], lhsT=redmat_wb[:], rhs=mb[:],
                                     start=False, stop=True)
                    nc.vector.tensor_copy(out=q0[0:64, sl], in_=po0[:])
                    # hs = [w2*sa; w2*sa]
                    hs = qp.tile([128, GW], BF, tag="hs")
                    nc.vector.tensor_scalar_mul(out=hs[0:64, :],
                                                in0=e[0:64, 0, :],
                                                scalar1=w2col[0:64, :])
                    nc.vector.tensor_scalar_mul(out=hs[64:128, :],
                                                in0=e[0:64, 0, :],
                                                scalar1=w2col[0:64, :])
                    # ov_k = (w2*sa)*tv_k + (w3*t0)*va_k
                    pa0 = qp.tile([128, GW], BF, tag="pa0")
                    pa1 = qp.tile([128, GW], BF, tag="pa1")
                    pb0 = qp.tile([128, GW], BF, tag="pb0")
                    pb1 = qp.tile([128, GW], BF, tag="pb1")
                    nc.vector.tensor_tensor(out=pa0[:], in0=tf0, in1=hs[:], op=MUL)
                    nc.vector.tensor_tensor(out=pa1[:], in0=tf1, in1=hs[:], op=MUL)
                    gps.tensor_tensor(out=pb0[:], in0=t0w[:], in1=e0, op=MUL)
                    gps.tensor_tensor(out=pb1[:], in0=t0w[:], in1=e1, op=MUL)
                    gps.tensor_tensor(out=q0[64:128, sl], in0=pa0[64:128, :],
                                      in1=pb0[64:128, :], op=ADD)
                    gps.tensor_tensor(out=q1[:, sl], in0=pa1[:], in1=pb1[:],
                                      op=ADD)
                    # ---- mlp0: x = [se-folded ; o0] ----
                    ph1 = pp.tile([128, 2, GW], F32, tag="mm")
                    for mbi in range(2):
                        msl = slice(mbi * 128, (mbi + 1) * 128)
                        nc.tensor.matmul(out=ph1[:, mbi, :], lhsT=l0w1se[:, msl],
                                         rhs=se3[:], start=True, stop=False)
                        nc.tensor.matmul(out=ph1[:, mbi, :], lhsT=l0w1b[:, msl],
                                         rhs=q0[0:64, sl], start=False, stop=True)
                    h1 = sp.tile([128, 2, GW], BF, tag="h1")
                    nc.scalar.activation(out=h1[:], in_=ph1[:], func=SILU)
                    ph2 = pp.tile([128, 2, GW], F32, tag="mm")
                    for mbi in range(2):
                        msl = slice(mbi * 128, (mbi + 1) * 128)
                        nc.tensor.matmul(out=ph2[:, mbi, :], lhsT=l0w2a[:, msl],
                                         rhs=h1[:, 0, :], start=True, stop=False)
                        nc.tensor.matmul(out=ph2[:, mbi, :], lhsT=l0w2b[:, msl],
                                         rhs=h1[:, 1, :], start=False, stop=True)
                    h2 = sp.tile([128, 2, GW], BF, tag="h2")
                    nc.scalar.activation(out=h2[:], in_=ph2[:], func=SILU)
                    # lat = h2 @ W3 : slot0 = acc1, slot1 = env_w2
                    pw3 = pp.tile([128, 2, GW], F32, tag="mm")
                    for mbi in range(2):
                        msl = slice(mbi * 128, (mbi + 1) * 128)
                        nc.tensor.matmul(out=pw3[:, mbi, :], lhsT=l0w3a[:, msl],
                                         rhs=h2[:, 0, :], start=True, stop=False)
                        nc.tensor.matmul(out=pw3[:, mbi, :], lhsT=l0w3b[:, msl],
                                         rhs=h2[:, 1, :], start=False, stop=True)
                    law = sp.tile([128, 2, GW], BF, tag="law")
                    nc.vector.tensor_copy(out=law[:], in_=pw3[:])
                    nc.sync.dma_start(out=acc1T[:, sl], in_=law[:, 0, :])
                    x2 = build_X(ppt, sp, law[:, 1, :], g)
                    for s in range(GRP):
                        scat2(g * GRP + s, m, x2, s)

            # chunked collective 2 + per-chunk table loads
            T2c = []
            for k in range(N_CHUNK):
                nc.gpsimd.collective_compute(
                    "AllReduce", ADD, replica_groups=rg,
                    ins=[t2loc[k * CA:(k + 1) * CA, :].opt()],
                    outs=[t2glob[k].opt()])
                tt = qpool.tile([128, CB, 256], BF, tag=f"T2c{k}")
                nc.sync.dma_start(
                    out=tt[:],
                    in_=t2glob[k][:].rearrange("(b p) f -> p b f", p=128))
                T2c.append(tt)

            # =================== Phase 5: gather-2, TP1, mlp1 ================
            with tc.tile_pool(name="p5sb", bufs=2) as sp, \
                 tc.tile_pool(name="p5ps", bufs=2, space="PSUM") as pp, \
                 tc.tile_pool(name="p5out", bufs=1, space="PSUM") as outp, \
                 tc.tile_pool(name="p5red", bufs=1, space="PSUM") as redp, \
                 tc.tile_pool(name="p5env", bufs=1, space="PSUM") as ep:
                for g in range(NG):
                    sl = slice(g * GW, (g + 1) * GW)
                    m2 = build_M2_eq(sp, g)
                    pe = ep.tile([128, 2, GW], F32, tag="env")
                    t = 0
                    while t < GRP:
                        b = (g * GRP + t) // t_b
                        n = 1
                        while t + n < GRP and (g * GRP + t + n) // t_b == b:
                            n += 1
                        ts = slice(t * 128, (t + n) * 128)
                        Tt = T2c[b // CB]
                        bb = b % CB
                        nc.tensor.matmul(out=pe[:, 0, ts], lhsT=Tt[:, bb, 0:128],
                                         rhs=m2[:, ts], start=True, stop=True)
                        nc.tensor.matmul(out=pe[:, 1, ts], lhsT=Tt[:, bb, 128:256],
                                         rhs=m2[:, ts], start=True, stop=True)
                        t += n
                    e = sp.tile([128, 2, GW], BF, tag="e")
                    nc.scalar.activation(out=e[:], in_=pe[:], func=COPY)
                    # o_last = redmat^T (e0*q0) + redmat^T (e1*q1)
                    pa = sp.tile([128, GW], BF, tag="pa")
                    pb = sp.tile([128, GW], BF, tag="pb")
                    nc.vector.tensor_tensor(out=pa[:], in0=e[:, 0, :],
                                            in1=q0[:, sl], op=MUL)
                    nc.vector.tensor_tensor(out=pb[:], in0=e[:, 1, :],
                                            in1=q1[:, sl], op=MUL)
                    pol = redp.tile([64, GW], F32, tag="pol")
                    nc.tensor.matmul(out=pol[:], lhsT=redmat[:], rhs=pa[:],
                                     start=True, stop=False)
                    nc.tensor.matmul(out=pol[:], lhsT=redmat[:], rhs=pb[:],
                                     start=False, stop=True)
                    ol = sp.tile([64, GW], BF, tag="ol")
                    nc.vector.tensor_copy(out=ol[:], in_=pol[:])
                    # mlp1: x = [se-folded ; acc1 ; o_last]
                    se5 = sp.tile([64, GW], BF, tag="se5")
                    nc.sync.dma_start(out=se5[:], in_=seT[:, sl])
                    ac1 = sp.tile([128, GW], BF, tag="ac1")
                    nc.sync.dma_start(out=ac1[:], in_=acc1T[:, sl])
                    ph1t = pp.tile([128, 2, GW], F32, tag="mm")
                    for mbi in range(2):
                        msl = slice(mbi * 128, (mbi + 1) * 128)
                        nc.tensor.matmul(out=ph1t[:, mbi, :], lhsT=l1w1se[:, msl],
                                         rhs=se5[:], start=True, stop=False)
                        nc.tensor.matmul(out=ph1t[:, mbi, :], lhsT=l1w1b[:, msl],
                                         rhs=ac1[:], start=False, stop=False)
                        nc.tensor.matmul(out=ph1t[:, mbi, :], lhsT=l1w1c[:, msl],
                                         rhs=ol[:], start=False, stop=True)
                    h1 = sp.tile([128, 2, GW], BF, tag="h1")
                    nc.scalar.activation(out=h1[:], in_=ph1t[:], func=SILU)
                    ph2t = pp.tile([128, 2, GW], F32, tag="mm")
                    for mbi in range(2):
                        msl = slice(mbi * 128, (mbi + 1) * 128)
                        nc.tensor.matmul(out=ph2t[:, mbi, :], lhsT=l1w2a[:, msl],
                                         rhs=h1[:, 0, :], start=True, stop=False)
                        nc.tensor.matmul(out=ph2t[:, mbi, :], lhsT=l1w2b[:, msl],
                                         rhs=h1[:, 1, :], start=False, stop=True)
                    h2 = sp.tile([128, 2, GW], BF, tag="h2")
                    nc.scalar.activation(out=h2[:], in_=ph2t[:], func=SILU)
                    pw3 = outp.tile([128, GW], F32, tag="out")
                    nc.tensor.matmul(out=pw3[:], lhsT=l1w3a[:], rhs=h2[:, 0, :],
                                     start=True, stop=False)
                    nc.tensor.matmul(out=pw3[:], lhsT=l1w3b[:], rhs=h2[:, 1, :],
                                     start=False, stop=True)
                    a2sb = sp.tile([128, GW], BF, tag="a2sb")
                    nc.vector.tensor_copy(out=a2sb[:], in_=pw3[:])
                    nc.sync.dma_start(out=acc2T[:, sl], in_=a2sb[:])

    nc.compile()
    return nc, dict(NT=NT, E_PAD=E_PAD, A=A, n_blk=n_blk, t_b=t_b)


# =====================================================================
# Host side
# =====================================================================

def _prep_core(centers_l, se_l, tf_l, basis_l, n_blk, t_b, w3vec):
    """Sort/pad one core's edges into the block-tile layout."""
    NT = n_blk * t_b
    E_PAD = NT * 128
    order = np.argsort(centers_l, kind="stable")
    bid = centers_l[order] // ABLK
    perm = np.full(E_PAD, -1, np.int64)
    for b in range(n_blk):
        run = order[bid == b]
        assert len(run) <= t_b * 128, f"block {b} overflow: {len(run)}"
        perm[b * t_b * 128: b * t_b * 128 + len(run)] = run
    valid = perm >= 0
    psafe = np.where(valid, perm, 0)

    idxrel = np.where(
        valid, centers_l[psafe] - ABLK * (np.arange(E_PAD) // (t_b * 128)),
        -1).astype(np.float32)

    se_pad = se_l[psafe] * valid[:, None]          # [E_PAD, 64]
    tf_pad = tf_l[psafe] * valid[:, None, None]    # [E_PAD, 64, 4]
    basis_pad = basis_l[psafe] * valid[:, None]    # [E_PAD, 4]

    seT = np.ascontiguousarray(se_pad.T).astype(BF_NP)
    # raw tensor features, feature-major: rows f = 64*comp+chan, two chunks
    tf_cm = np.ascontiguousarray(
        tf_pad.transpose(2, 1, 0)).reshape(4 * 64, E_PAD).astype(np.float32)
    tfT = np.ascontiguousarray(
        np.stack([tf_cm[0:128], tf_cm[128:256]])).astype(BF_NP)
    t0wT = np.ascontiguousarray(
        np.tile(tf_cm[0:64] * w3vec[:, None], (2, 1))).astype(BF_NP)
    basis_pm = np.ascontiguousarray(
        basis_pad.reshape(NT, 128, 4).transpose(1, 0, 2)).astype(BF_NP)
    idxcol = np.ascontiguousarray(
        idxrel.reshape(NT, 128).T).astype(BF_NP)           # [128, NT]
    idxbc = np.ascontiguousarray(
        np.tile(idxrel.astype(BF_NP)[None, :], (128, 1)))  # [128, E_PAD]
    return dict(seT=seT, tfT=tfT, t0wT=t0wT, basis_pm=basis_pm,
                idxcol=idxcol, idxbc=idxbc), perm


def _weights_maps(W_proj, w_tp0, w_tp1, l0, l1):
    bf = lambda a: np.ascontiguousarray(a).astype(BF_NP)
    f32 = np.float32
    w0 = (w_tp0[0] * NORM).astype(f32)
    w1 = (w_tp0[1] * NORM * INV_SQRT3).astype(f32)
    w2 = (w_tp0[2] * NORM).astype(f32)
    w3 = (w_tp0[3] * NORM).astype(f32)
    wx = np.concatenate([w_tp1[0] * NORM,
                         w_tp1[1] * NORM * INV_SQRT3]).astype(f32)  # [128]
    eye64 = np.eye(64, dtype=f32)
    redmat = np.tile(eye64, (2, 1))
    redmat_wa = np.concatenate([eye64 * w0, eye64 * w1], 0)   # [128, 64]
    redmat_wb = np.concatenate([eye64 * w1, eye64 * w1], 0)
    l0w1, l0w2, l0w3 = l0
    l0w3 = l0w3.copy()
    l0w3[:, 128:256] = l0w3[:, 128:256] * wx[None, :]
    l1w1, l1w2, l1w3 = l1
    Wp = np.asarray(W_proj, f32)
    l0w1se = Wp[:, 0:128] @ l0w1[0:128]       # [64, 256]
    l1w1se = Wp[:, 0:128] @ l1w1[0:128]       # [64, 256]
    return {
        "iota_part": bf(np.repeat(np.arange(128, dtype=f32)[:, None], 512, 1)),
        "iota_tile": bf(np.tile(np.arange(128, dtype=f32)[None, :], (128, 4))),
        "eye": bf(np.eye(128, dtype=f32)),
        "redmat": bf(redmat),
        "redmat_wa": bf(redmat_wa),
        "redmat_wb": bf(redmat_wb),
        "w2col": np.ascontiguousarray(np.tile(w2, 2)[:, None], f32),
        "w3col": np.ascontiguousarray(np.tile(w3, 2)[:, None], f32),
        "wproj": bf(Wp),
        "l0w1se": bf(l0w1se), "l0w1b": bf(l0w1[128:192]),
        "l0w2a": bf(l0w2[0:128]), "l0w2b": bf(l0w2[128:256]),
        "l0w3a": bf(l0w3[0:128]), "l0w3b": bf(l0w3[128:256]),
        "l1w1se": bf(l1w1se), "l1w1b": bf(l1w1[128:256]),
        "l1w1c": bf(l1w1[256:320]),
        "l1w2a": bf(l1w2[0:128]), "l1w2b": bf(l1w2[128:256]),
        "l1w3a": bf(l1w3[0:128]), "l1w3b": bf(l1w3[128:256]),
    }


_CACHE = {}


def kernel(edge_index, num_atoms, tensor_basis, tensor_features, scalar_embed,
           W_proj, w_tp0, w_tp1,
           lat0_W1, lat0_W2, lat0_W3, lat1_W1, lat1_W2, lat1_W3,
           _trace=False, _tmpdir=None):
    if "nc" not in _CACHE:
        _CACHE["nc"], _CACHE["meta"] = build_graph()
    nc, meta = _CACHE["nc"], _CACHE["meta"]
    n_blk, t_b, E_PAD = meta["n_blk"], meta["t_b"], meta["E_PAD"]

    edge_index = np.asarray(edge_index)
    centers = edge_index[0]
    tb = np.asarray(tensor_basis, np.float32)
    tf = np.asarray(tensor_features, np.float32)
    se = np.asarray(scalar_embed, np.float32)

    wmaps = _weights_maps(
        np.asarray(W_proj, np.float32), np.asarray(w_tp0, np.float32),
        np.asarray(w_tp1, np.float32),
        (np.asarray(lat0_W1, np.float32), np.asarray(lat0_W2, np.float32),
         np.asarray(lat0_W3, np.float32)),
        (np.asarray(lat1_W1, np.float32), np.asarray(lat1_W2, np.float32),
         np.asarray(lat1_W3, np.float32)))

    w3vec = np.asarray(w_tp0, np.float32)[3] * NORM
    in_maps, perms = [], []
    for c in range(N_CORES):
        s = slice(c * E_LOC, (c + 1) * E_LOC)
        m, perm = _prep_core(centers[s], se[s], tf[s], tb[s], n_blk, t_b,
                             w3vec)
        m.update(wmaps)
        in_maps.append(m)
        perms.append(perm)

    res = run_bass_kernel_spmd(nc, in_maps, core_ids=list(range(N_CORES)),
                               trace=_trace, tmpdir=_tmpdir)
    out = np.empty((E, NS * 3), np.float32)
    for c in range(N_CORES):
        r = res.results[c]
        op = np.concatenate(
            [np.asarray(r["acc0T"]).astype(np.float32).T,
             np.asarray(r["acc1T"]).astype(np.float32).T,
             np.asarray(r["acc2T"]).astype(np.float32).T], axis=1)  # [E_PAD,384]
        perm = perms[c]
        valid = perm >= 0
        out[c * E_LOC + perm[valid]] = op[valid]
    if _trace:
        kernel.last_exec_time_ns = res.exec_time_ns
        if res.instructions_and_trace:
            kernel.last_trace = res.instructions_and_trace[1]
    return out
